# revision 41
# baseline (speedup 1.0000x reference)
"""AtomwiseReadout segment-reduce kernel for 8 TRN2 NeuronCores — v8.

reference computation:
    atomwise = f @ w_e + z_bias[z]            # [N, 1]
    e_total  = segment_sum(atomwise, seg)     # [B, 1], 20 atoms per molecule

The kernel is a pure HBM stream (128.45 MB of f32 f per core, read once);
everything else is organized so the SWDGE cast-DMA stream never waits:

  - atoms sharded contiguously at molecule boundaries across 8 cores; each
    core runs 98 "supers" of 2560 atoms (= 128 molecules).  Within a super,
    partition p holds atoms [20p, 20p+20) = exactly molecule p, so every
    partition's f data is ONE contiguous 10 KB chunk in HBM -> 10 KB DMA
    descriptors, ~380 ns M2S each, the 16-SDMA read path saturates at
    ~427 GB/s (uncontended) / ~358 (stack shared with the paired core).
  - f streams in B=2-super cast-DMA batches (2.6 MB, f32->bf16).  A 1-super
    head batch halves the Q7 descriptor-emission latency in front of the
    first byte.  B=2 also keeps the PE's inter-batch idle under the ~5.2us
    HAM re-throttle window.  8 f-buffers absorb consumer jitter.
  - per batch the PE sums the 20 atom slices with accumulating identity
    matmuls into PSUM; the DVE then does S*w mult + reduce -> res[p, super].
  - z_bias[z] is an 85-pass value sweep on the DVE (replaces a one-hot/
    histogram that kept DVE 85% + Scalar 42% busy and starved the DMA ring):
        for v in 1..85:  m = (z == v) * zb[v]        (tensor_scalar, 4x bf16)
                         acc[v%2] += m               (tensor_tensor, 2x bf16)
    Each z matches exactly one v, so every acc element is assigned once and
    otherwise accumulates zeros -> the sweep is EXACT in bf16.  Alternating
    accumulators break the DVE output-dependence chain.  Passes are emitted
    2 per batch so they fill DVE idle without delaying the flushes.
  - w_e ships once ([128, 128], 64 KB) and is tiled x20 on-device so const
    DMAs do not steal prologue stream bandwidth.
  - the last N_TAIL=3 supers flush via fused scalar_tensor_tensor dots on
    the DVE instead of the PE: the last PE batch's cold 20-matmul chain
    needs ~9us after its f arrives, so the PE drains while the DVE-tail
    supers stream.  At super 95 the bias total (one reduce over acc0+acc1)
    folds in and res2[:, :95] DMAs out; the very last super arrives as two
    half-DMAs so only ~1.5us of fused dot + one 512 B store remain after
    the final f byte.

Measured (core 0): ~326 us when this core wins its HBM-stack arbitration
race, ~385-400 us when its partner core wins (the pair shares 716 GB/s;
arbitration is winner-take-most and run-random; see STAGGER note below).
rel err vs the f32 reference: 1.8e-3 (from the f32->bf16 f cast).
"""

import numpy as np
import ml_dtypes

import concourse.bass as bass
import concourse.bacc as bacc
import concourse.mybir as mybir
import concourse.tile as tile
from concourse.bass_utils import run_bass_kernel_spmd


def _ensure_ntff_hook():
    """Restore the NTFF profile hook if the image's antenv lacks axon_hooks.

    trn_boot.boot() registers this hook at interpreter start, but degrades
    silently when ``antenv.axon_hooks`` is missing — and bass_utils then
    crashes on the import when trace=True. Recreate the module with the
    same hook boot() would have installed. No-op when the real module
    exists.
    """
    try:
        import antenv.axon_hooks  # noqa: F401

        return
    except ImportError:
        pass
    try:
        import sys
        import types

        from trn_agent_boot.trn_boot import _ntff_profile_via_ctypes

        hook = _ntff_profile_via_ctypes("/opt/axon/libaxon_pjrt.so")
        mod = types.ModuleType("antenv.axon_hooks")
        mod.get_axon_ntff_profile_hook = lambda: hook
        mod.set_axon_ntff_profile_hook = lambda h: None
        sys.modules["antenv.axon_hooks"] = mod
    except Exception:
        pass


_ensure_ntff_hook()

# problem constants (hardcoded per spec)
N_ATOMS = 2_000_000
N_MOL = 100_000
APM = 20          # atoms per molecule
D = 128           # feature dim
V = 86            # z vocabulary (0..85); z values are in [1, 85]
N_CORES = 8

# tiling
P = 128                       # partitions
SUP_ATOMS = P * APM           # 2560 atoms per super (1 molecule/partition)
SUP_MOLS = P                  # 128 molecules per super
N_SUP = 98                    # supers per core
B = 2                         # supers per DMA/PSUM batch
SHARD_ATOMS = N_SUP * SUP_ATOMS   # 250880
SHARD_MOLS = SHARD_ATOMS // APM   # 12544
MOLS_PER_CORE = N_MOL // N_CORES  # 12500

F32 = mybir.dt.float32
BF16 = mybir.dt.bfloat16

TRACE = False  # test harness can flip this to get a profile
FBUFS = 8      # f-tile double buffering depth (10 KB/partition each)
PRE_PASSES = 8   # bias sweep passes emitted before the batch loop
PASSES_PER_BATCH = 2
STAGGER = False  # stagger experiments inconclusive; see transcript


N_TAIL = 3  # trailing supers flushed on the DVE (fused dot) instead of the
# PE: the last PE batch's cold 20-matmul chain takes ~9us after its f
# arrives, so with a shorter tail it finishes AFTER the stream ends and
# gates the result ship (seen in the trace: flush at stream_end+2.6us).
# Three DVE-tail supers let the PE drain ~11us before the last byte.


def _batches(n_sup):
    """SWDGE batch plan for supers [0, n_sup-1): a 1-super head batch (128
    descriptors emit in ~1.5us, so first bytes hit the wire early), B-sized
    body, then 1-super tail batches consumed by DVE fused dots. The last
    super is not in the list — it streams as two half-DMAs so the final
    flush only waits on the trailing 5 KB/partition."""
    out = [(0, 1)]
    out += [(s, B) for s in range(1, n_sup - N_TAIL, B)]
    out += [(n_sup - 3, 1), (n_sup - 2, 1)]
    return out


def build(nc, n_sup=N_SUP):
    shard_atoms = n_sup * SUP_ATOMS
    batches = _batches(n_sup)
    za = n_sup * APM  # z elements per partition

    f = nc.dram_tensor("f", [shard_atoms, D], F32, kind="ExternalInput")
    zc = nc.dram_tensor("z_cols", [P, za], BF16, kind="ExternalInput")
    ident = nc.dram_tensor("ident", [P, P], BF16, kind="ExternalInput")
    w = nc.dram_tensor("w_rep", [P, D], F32, kind="ExternalInput")
    zbb = nc.dram_tensor("zb_bcast", [P, V], F32, kind="ExternalInput")
    out = nc.dram_tensor("out", [P, n_sup], F32, kind="ExternalOutput")

    # atom row = n*2560 + p*20 + a  ->  [p, n, (a d)]: per (p, n) the HBM
    # data is one contiguous 20*128*4B = 10 KB run.
    fv = f.ap().rearrange("(n p a) d -> p n (a d)", p=P, a=APM)

    with tile.TileContext(nc) as tc:
        with (
            tc.tile_pool(name="const", bufs=1) as cpool,
            tc.tile_pool(name="fpool", bufs=FBUFS) as fpool,
            tc.tile_pool(name="work", bufs=2) as pool,
            tc.tile_pool(name="psum_s", bufs=4, space="PSUM") as ppool_s,
        ):
            # Stagger: paired NeuronCores share one HBM stack and the
            # arbitration race is sticky — the core that saturates its DMA
            # queues first sustains ~427 GB/s while its partner gets the
            # remainder until the winner drains.  Give core 0 the head
            # start deterministically: every other core prepends a
            # throwaway ~12us read (skipped on core 0 via cond).
            if STAGGER:
                pid = nc.gpsimd.partition_id()
                # ap_or_oob requires cond in [0, 1].  Empirically the HBM
                # arbitration favors the LATE joiner (deficit-based): when
                # the partners were delayed, core 0 lost 5/5.  So delay
                # core 0 itself.
                notzero = nc.gpsimd.scalar_reg_alu(
                    mybir.AluOpType.is_equal, pid, 0
                )
                # descriptor-bound throwaway: 1024 descriptors of 64 B keep
                # the partner's Q7/SWDGE busy ~10us while reading only 64 KB
                # of HBM, so core 0's stream gets a genuine head start
                stag_sb = cpool.tile([P, 8 * 16], F32)
                stag_src = fv[:, 40, :].rearrange("p (k r) -> p k r", k=8)
                nc.gpsimd.dma_start(
                    out=stag_sb[:].rearrange("p (k r) -> p k r", k=8),
                    in_=stag_src[:, :, :16],
                    cond=notzero,
                    cond_hint=True,
                )

            # SWDGE cast-DMA stream for supers 1..n_sup-1
            f_tiles = {}

            def emit_fdma(bi):
                sup0, nb = batches[bi]
                f_sb = fpool.tile([P, B * SUP_ATOMS], BF16, tag="f")
                nc.gpsimd.dma_start(
                    out=f_sb[:, : nb * SUP_ATOMS],
                    in_=fv[:, sup0 : sup0 + nb, :],
                )
                f_tiles[bi] = f_sb

            emit_fdma(0)

            w_sb = cpool.tile([P, D], F32)
            nc.sync.dma_start(out=w_sb[:], in_=w.ap())
            zc_sb = cpool.tile([P, za], BF16)
            nc.sync.dma_start(out=zc_sb[:], in_=zc.ap())
            ident_sb = cpool.tile([P, P], BF16)
            nc.sync.dma_start(out=ident_sb[:], in_=ident.ap())
            zbb_sb = cpool.tile([P, V], F32)
            nc.sync.dma_start(out=zbb_sb[:], in_=zbb.ap())

            # DVE-local copies so the sweep/flush never re-wait DMA sems;
            # w is shipped once (64 KB) and tiled x20 on-device so the
            # const DMAs do not steal prologue stream bandwidth
            zc2 = cpool.tile([P, za], BF16)
            nc.vector.tensor_copy(out=zc2[:], in_=zc_sb[:])
            w20_sb = cpool.tile([P, APM * D], F32)
            nc.vector.tensor_copy(
                out=w20_sb[:].rearrange("p (a d) -> p a d", a=APM),
                in_=w_sb[:].unsqueeze(1).to_broadcast([P, APM, D]),
            )
            w2 = w20_sb
            zbb2 = cpool.tile([P, V], F32)
            nc.vector.tensor_copy(out=zbb2[:], in_=zbb_sb[:])

            res = cpool.tile([P, n_sup], F32)
            res2 = cpool.tile([P, n_sup], F32)
            bias_tot = cpool.tile([P, n_sup], F32)
            acc = [
                cpool.tile([P, za], BF16, name=f"acc{i}") for i in range(2)
            ]
            mt = [cpool.tile([P, za], BF16, name=f"mt{i}") for i in range(2)]

            # ---- bias value sweep: acc[v%2] += (z == v) * zb[v] ----
            vs = list(range(1, V))  # z values are 1..85
            pass_state = {"k": 0}

            def emit_pass():
                k = pass_state["k"]
                if k >= len(vs):
                    return
                pass_state["k"] = k + 1
                v = vs[k]
                a = acc[k % 2]
                with nc.allow_low_precision(
                    reason="each z matches exactly one v; all other adds are +0"
                ):
                    if k < 2:
                        # first write of this accumulator
                        nc.vector.tensor_scalar(
                            out=a[:], in0=zc2[:],
                            scalar1=float(v), scalar2=zbb2[:, v : v + 1],
                            op0=mybir.AluOpType.is_equal,
                            op1=mybir.AluOpType.mult,
                        )
                    else:
                        m = mt[k % 2]
                        nc.vector.tensor_scalar(
                            out=m[:], in0=zc2[:],
                            scalar1=float(v), scalar2=zbb2[:, v : v + 1],
                            op0=mybir.AluOpType.is_equal,
                            op1=mybir.AluOpType.mult,
                        )
                        nc.vector.tensor_tensor(
                            out=a[:], in0=a[:], in1=m[:],
                            op=mybir.AluOpType.add,
                        )

            # the last super flushes on the DVE via fused dots
            split = n_sup - N_TAIL
            junk = cpool.tile([P, SUP_ATOMS], F32)

            for _ in range(PRE_PASSES):
                emit_pass()

            for bi, (sup0, nb) in enumerate(batches):
                if bi > 0:
                    emit_fdma(bi)
                f_sb = f_tiles.pop(bi)

                if sup0 >= split:
                    # tail super: one fused dot on the (long idle) DVE —
                    # accum = sum_over_free((f * 1.0) * w20) = molecule
                    # energy; drains ~2.7us instead of a cold 20-matmul
                    # PE chain.
                    nc.vector.scalar_tensor_tensor(
                        out=junk[:],
                        in0=f_sb[:, :SUP_ATOMS],
                        scalar=1.0,
                        in1=w20_sb[:],
                        op0=mybir.AluOpType.mult,
                        op1=mybir.AluOpType.mult,
                        accum_out=res[:, sup0 : sup0 + 1],
                    )
                    nc.vector.tensor_tensor(
                        out=res2[:, sup0 : sup0 + 1],
                        in0=res[:, sup0 : sup0 + 1],
                        in1=bias_tot[:, sup0 : sup0 + 1],
                        op=mybir.AluOpType.add,
                    )
                    nc.sync.dma_start(
                        out=out.ap()[:, sup0 : sup0 + 1],
                        in_=res2[:, sup0 : sup0 + 1],
                    )
                    continue

                # [p, (j a d)] -> [p, a, j, d] so each a-slice is a matmul rhs
                fva = f_sb[:].rearrange("p (j a d) -> p a j d", a=APM, d=D)

                s_ps = ppool_s.tile([P, B * D], F32, tag="S")
                for a in range(APM):
                    nc.tensor.matmul(
                        out=s_ps[:, : nb * D],
                        lhsT=ident_sb[:],
                        rhs=fva[:, a : a + 1, :nb, :],
                        start=(a == 0),
                        stop=(a == APM - 1),
                    )

                if sup0 + nb == split:
                    # last PE batch: its cold 20-matmul chain finishes only
                    # ~1us before the stream ends, so DEFER its flush until
                    # after the tail-super dots are emitted — otherwise they
                    # queue behind it on the DVE FIFO and run post-stream.
                    # The bias total only needs the (long done) sweep, so
                    # emit it now for the tail supers' adds.
                    assert pass_state["k"] == len(vs)
                    asum = mt[0]  # sweep scratch is dead now; reuse
                    nc.vector.tensor_tensor(
                        out=asum[:], in0=acc[0][:], in1=acc[1][:],
                        op=mybir.AluOpType.add,
                    )
                    nc.vector.tensor_reduce(
                        out=bias_tot[:],
                        in_=asum[:].rearrange("p (n a) -> p n a", a=APM),
                        axis=mybir.AxisListType.X,
                        op=mybir.AluOpType.add,
                    )
                    deferred = (s_ps, sup0, nb)
                    continue

                scr = pool.tile([P, B * D], F32, tag="scr")
                nc.vector.tensor_tensor(
                    out=scr[:, : nb * D],
                    in0=s_ps[:, : nb * D],
                    in1=w2[:, : nb * D],
                    op=mybir.AluOpType.mult,
                )
                nc.vector.tensor_reduce(
                    out=res[:, sup0 : sup0 + nb],
                    in_=scr[:].rearrange("p (j d) -> p j d", d=D)[:, :nb, :],
                    axis=mybir.AxisListType.X,
                    op=mybir.AluOpType.add,
                )
                for _ in range(PASSES_PER_BATCH):
                    emit_pass()

            # deferred flush of the last PE batch, then the bulk result ship
            d_ps, d_sup0, d_nb = deferred
            scr = pool.tile([P, B * D], F32, tag="scr")
            nc.vector.tensor_tensor(
                out=scr[:, : d_nb * D],
                in0=d_ps[:, : d_nb * D],
                in1=w2[:, : d_nb * D],
                op=mybir.AluOpType.mult,
            )
            nc.vector.tensor_reduce(
                out=res[:, d_sup0 : d_sup0 + d_nb],
                in_=scr[:].rearrange("p (j d) -> p j d", d=D)[:, :d_nb, :],
                axis=mybir.AxisListType.X,
                op=mybir.AluOpType.add,
            )
            nc.vector.tensor_tensor(
                out=res2[:, :split], in0=res[:, :split],
                in1=bias_tot[:, :split], op=mybir.AluOpType.add,
            )
            nc.sync.dma_start(out=out.ap()[:, :split], in_=res2[:, :split])

            # final super streams as two half-DMAs so the last fused dot
            # only waits on the trailing 5 KB/partition
            sl = n_sup - 1
            half = SUP_ATOMS // 2
            fl_sb = fpool.tile([P, SUP_ATOMS], BF16, tag="f")
            nc.gpsimd.dma_start(out=fl_sb[:, :half], in_=fv[:, sl : sl + 1, :half])
            nc.gpsimd.dma_start(out=fl_sb[:, half:], in_=fv[:, sl : sl + 1, half:])
            eh = pool.tile([P, 3], F32, tag="eh")
            for h in range(2):
                nc.vector.scalar_tensor_tensor(
                    out=junk[:, :half],
                    in0=fl_sb[:, h * half : (h + 1) * half],
                    scalar=1.0,
                    in1=w20_sb[:, h * half : (h + 1) * half],
                    op0=mybir.AluOpType.mult,
                    op1=mybir.AluOpType.mult,
                    accum_out=eh[:, h : h + 1],
                )
                if h == 0:
                    # fold bias + first half while the second half streams,
                    # leaving a single add on the post-stream critical path
                    nc.vector.tensor_tensor(
                        out=eh[:, 2:3], in0=eh[:, 0:1],
                        in1=bias_tot[:, sl : sl + 1],
                        op=mybir.AluOpType.add,
                    )
            nc.vector.tensor_tensor(
                out=res2[:, sl : sl + 1], in0=eh[:, 2:3], in1=eh[:, 1:2],
                op=mybir.AluOpType.add,
            )
            nc.sync.dma_start(
                out=out.ap()[:, sl : sl + 1], in_=res2[:, sl : sl + 1]
            )
    nc.compile()
    return nc


def _prep_core_inputs(f, z, w_e, z_bias, start, n_sup=N_SUP):
    """Per-core input map. f/z are the full arrays; start = first atom row."""
    shard_atoms = n_sup * SUP_ATOMS
    zs = np.asarray(z[start : start + shard_atoms]).astype(np.float32)
    # z_cols[p, n*20+a] = z[start + n*2560 + 20p + a]
    z_cols = np.ascontiguousarray(
        zs.reshape(n_sup, P, APM).transpose(1, 0, 2).reshape(P, n_sup * APM)
    ).astype(ml_dtypes.bfloat16)
    return {
        "f": f[start : start + shard_atoms],
        "z_cols": z_cols,
        "ident": _IDENT,
        "w_rep": np.ascontiguousarray(
            np.broadcast_to(np.asarray(w_e, np.float32).reshape(1, D), (P, D))
        ),
        "zb_bcast": np.ascontiguousarray(
            np.broadcast_to(
                np.asarray(z_bias, np.float32).reshape(1, V), (P, V)
            )
        ),
    }


_IDENT = np.eye(P, dtype=np.float32).astype(ml_dtypes.bfloat16)

_NC_CACHE = {}
_LAST_RESULTS = None  # BassKernelResults of the most recent run (for profiling)


def kernel(z, f, num_atoms, w_e, z_bias):
    global _LAST_RESULTS
    z = np.asarray(z)
    f = np.ascontiguousarray(np.asarray(f, dtype=np.float32))
    w_e = np.asarray(w_e, dtype=np.float32)
    z_bias = np.asarray(z_bias, dtype=np.float32)
    assert f.shape == (N_ATOMS, D)

    key = ("v10", N_SUP, B, FBUFS, STAGGER)
    if key not in _NC_CACHE:
        _NC_CACHE[key] = build(bacc.Bacc(), N_SUP)
    nc = _NC_CACHE[key]

    # core i handles molecules [i*12500, (i+1)*12500); its shard starts at
    # atom i*250000 except the last core, whose shard is right-aligned so
    # no padding is ever needed.
    starts = [i * MOLS_PER_CORE * APM for i in range(N_CORES - 1)]
    starts.append(N_ATOMS - SHARD_ATOMS)
    in_maps = [_prep_core_inputs(f, z, w_e, z_bias, s) for s in starts]

    res = run_bass_kernel_spmd(nc, in_maps, core_ids=list(range(N_CORES)), trace=TRACE)
    _LAST_RESULTS = res

    out = np.empty((N_MOL, 1), np.float32)
    for i in range(N_CORES):
        # device layout: out[p, n] = e of molecule n*128 + p (within shard)
        arr = np.asarray(res.results[i]["out"])  # [P, n_sup]
        e = arr.T.reshape(N_SUP * SUP_MOLS)
        first_mol = starts[i] // APM
        lo = i * MOLS_PER_CORE
        out[lo : lo + MOLS_PER_CORE, 0] = e[lo - first_mol : lo - first_mol + MOLS_PER_CORE]
    return out
